# revision 19
# baseline (speedup 1.0000x reference)
"""MoE pointwise conv2d kernel for Trainium2 (8 NeuronCores, SPMD data-parallel).

Problem: out[b,o,h,w] = sum_i (sum_e routing[b,e] * weight[e,o,i]) * x[b,i,h,w]
Shapes:  x [64,384,28,28] f32, routing [64,8] f32, weight [8,384,384] f32.

Strategy (per core, 8 samples, all-fp16 wire + math, fp32 PSUM):
  - Routing combine runs on the TENSOR engine as one K=128 matmul:
    out[(b,s'), n] = sum_{(e,s)} S[(e,s),(b,s')] * W2[(e,s), n], where the
    host-built selector S[(e,s),(b,s')] = r[b,e] * (s==s') packs all 8
    samples x 16 o-phases into the 128 output partitions, and W2 is the
    expert weights relaid as [(e,s), (ki, t, i)] with o = t*16+s.
    This replaces 64 fused DVE MACs (~1.34us each, measured: the fused
    scalar_tensor_tensor form runs in 1x DVE mode) with ~9216 PE cycles.
  - DVE (2/3) + ScalarE (1/3) evacuate combine PSUM to SBUF fp16 (aggP);
    one XBAR DMA transpose per k-tile turns aggP[(b,s), (ki,t,i)] into
    aggT[i, (ki,t,(b,s))] (14ns per 16x128 tile, off all compute
    engines); per-(b,ki) DVE copies (4x mode) then gather the
    single-stride per-sample lhsT tiles walrus requires.
  - Per-sample GEMM out[b] = agg_b @ x_b on TensorE: lhsT is an
    [i][dt:8][s:16] strided view of aggT (o = 128*mo + dt*16 + s), 3
    k-tiles accumulated per PSUM bank; two 392-col halves of each mo run
    as interleaved groups in a [128,1024] 2-bank tile; one ScalarE copy
    evacuates both; outputs DMA per 2 samples.
  - DMA split: SP queue: sel, W2 head chunk, x0-3, XBAR transposes;
    Act queue: W2 tail, x4-7, outputs.
"""
import sys

sys.path.insert(0, "/opt/trn_rl_repo")

import numpy as np
from contextlib import ExitStack

B, C_IN, C_OUT, E, H, W = 64, 384, 384, 8, 28, 28
HW = H * W            # 784
N_CORES = 8
BPC = B // N_CORES    # 8 samples per core
KI = C_IN // 128      # 3 k-tiles
MO = C_OUT // 128     # 3 output-partition tiles
NCH = HW // 2         # 392 columns per matmul (psum group)
TT = 24               # o = t*16 + s, t in [0,24), s in [0,16)
SS = 16
WCOL2 = KI * TT * 128  # 9216 combine columns
NCK = 18               # combine psum chunks of 512

_cache = {}


def _build(reps=1, serialize_reps=False):
    import concourse.tile as tile
    import concourse.mybir as mybir
    from concourse import bacc
    from concourse.tile import add_dep_helper

    f32 = mybir.dt.float32
    f16 = mybir.dt.float16

    nc = bacc.Bacc("TRN2", target_bir_lowering=False, debug=False)
    x_d = nc.dram_tensor("x", [BPC, C_IN, HW], f16, kind="ExternalInput")
    sel_d = nc.dram_tensor("sel", [128, 128], f16, kind="ExternalInput")
    wt2_d = nc.dram_tensor("wt2", [128, WCOL2], f16, kind="ExternalInput")
    out_d = nc.dram_tensor("out", [reps * BPC, C_OUT, HW], f16,
                           kind="ExternalOutput")

    with tile.TileContext(nc) as tc:
        with ExitStack() as ctx:
            w2_pool = ctx.enter_context(tc.tile_pool(name="w2", bufs=2))
            sel_pool = ctx.enter_context(tc.tile_pool(name="sel", bufs=2))
            aggp_pool = ctx.enter_context(tc.tile_pool(name="agp", bufs=2))
            aggt_pool = ctx.enter_context(tc.tile_pool(name="agt", bufs=2))
            aggl_pool = ctx.enter_context(tc.tile_pool(name="agl", bufs=8))
            x_pool = ctx.enter_context(tc.tile_pool(name="xp", bufs=6))
            out_pool = ctx.enter_context(tc.tile_pool(name="op", bufs=3))
            psc_pool = ctx.enter_context(tc.tile_pool(name="psc", bufs=2,
                                                      space="PSUM"))
            psg_pool = ctx.enter_context(tc.tile_pool(name="psg", bufs=3,
                                                      space="PSUM"))

            prev_out_dmas, cur_out_dmas = [], []

            def _fence(inst):
                if serialize_reps:
                    for d in prev_out_dmas:
                        add_dep_helper(inst.ins, d.ins, reason="serialize reps")
                return inst

            for rep in range(reps):
                prev_out_dmas, cur_out_dmas = cur_out_dmas, []

                # ---- head DMAs ----
                sel_sb = sel_pool.tile([128, 128], f16)
                _fence(nc.sync.dma_start(sel_sb[:], sel_d[:]))
                w2_sb = w2_pool.tile([128, WCOL2], f16)
                _fence(nc.sync.dma_start(w2_sb[:, 0:3072], wt2_d[:, 0:3072]))
                _fence(nc.scalar.dma_start(w2_sb[:, 3072:WCOL2],
                                           wt2_d[:, 3072:WCOL2]))
                # x0/x1 up front on SP; x2/x3 are emitted AFTER the XBAR
                # transposes so the XBARs aren't queued behind ~6us of x
                # transfers on the in-order SP HWDGE queue; x4-7 ride Act.
                x_sb = [None] * BPC
                for b in range(BPC):
                    x_sb[b] = x_pool.tile([128, KI * HW], f16, name="xt")
                for b in (0, 1):
                    _fence(nc.sync.dma_start(
                        x_sb[b][:],
                        x_d[b].rearrange("(k p) c -> p k c", k=KI)))
                for b in range(4, BPC):
                    _fence(nc.scalar.dma_start(
                        x_sb[b][:],
                        x_d[b].rearrange("(k p) c -> p k c", k=KI)))

                # ---- combine: S.T @ W2 on TensorE, DVE/Pool evac to aggP ----
                aggp_sb = aggp_pool.tile([128, WCOL2], f16)
                for c in range(NCK):
                    psc = psc_pool.tile([128, 512], f32, name="psct")
                    nc.tensor.matmul(psc[:], sel_sb[:],
                                     w2_sb[:, c * 512:(c + 1) * 512],
                                     start=True, stop=True)
                    dst = aggp_sb[:, c * 512:(c + 1) * 512]
                    if c % 3:
                        nc.vector.tensor_copy(dst, psc[:])
                    else:
                        nc.scalar.copy(dst, psc[:])

                # ---- XBAR block-transpose per k-tile: aggP -> aggT ----
                # (per-sample 16-partition slabs are not 32-aligned, so
                # transpose all 128 partitions per ki, then DVE-permute)
                aggt_sb = aggt_pool.tile([128, WCOL2], f16)
                for ki in range(KI):
                    sl = slice(ki * TT * 128, (ki + 1) * TT * 128)
                    nc.sync.dma_start_transpose(
                        aggt_sb[:, sl].rearrange("p (t i) -> p t i", t=TT),
                        aggp_sb[:, sl])

                for b in (2, 3):
                    _fence(nc.sync.dma_start(
                        x_sb[b][:],
                        x_d[b].rearrange("(k p) c -> p k c", k=KI)))

                aggt_v = aggt_sb[:].rearrange("p (k t c) -> p k t c",
                                              k=KI, t=TT)

                # ---- DVE permute: per-sample contiguous lhsT tiles ----
                # aggT[i,(ki,t,(b,s))] -> aggL_b[i, ki*384 + t*16 + s]
                # (walrus requires a single-stride free dim on matmul
                # weights). Split per (b, ki) so each GEMM k-step only
                # waits for its own k-tile's XBAR.
                aggl = [None] * BPC
                for b in range(BPC):
                    aggl[b] = aggl_pool.tile([128, KI * C_OUT], f16,
                                             name="aglt")
                for ki in range(KI):
                    for b in range(BPC):
                        nc.vector.tensor_copy(
                            aggl[b][:, ki * C_OUT:(ki + 1) * C_OUT],
                            aggt_v[:, ki, :, b * SS:(b + 1) * SS])

                # ---- per-sample GEMM + evac + out DMA ----
                o_sb = None
                for b in range(BPC):
                    if b % 2 == 0:
                        o_sb = out_pool.tile([128, 2 * MO * HW], f16)
                    for mo in range(MO):
                        ps = psg_pool.tile([128, 1024], f32, name="psgt")
                        for ki in range(KI):
                            lhs = aggl[b][:, ki * C_OUT + mo * 128:
                                          ki * C_OUT + (mo + 1) * 128]
                            for n in range(2):
                                rhs = x_sb[b][:, ki * HW + n * NCH:
                                              ki * HW + (n + 1) * NCH]
                                nc.tensor.matmul(
                                    ps[:, n * 512:n * 512 + NCH], lhs, rhs,
                                    start=(ki == 0), stop=(ki == KI - 1),
                                )
                        dst = o_sb[:, (b % 2) * MO * HW + mo * HW:
                                   (b % 2) * MO * HW + (mo + 1) * HW]
                        nc.scalar.copy(
                            dst.rearrange("p (g c) -> p g c", g=2),
                            ps[:].rearrange("p (g c) -> p g c", g=2)
                              [:, :, 0:NCH],
                        )
                        if b == BPC - 1:
                            # last sample: per-mo DMA right after its evac
                            # shrinks the tail (and releases the rep fence
                            # sooner in the bench chain)
                            cur_out_dmas.append(nc.scalar.dma_start(
                                out_d[rep * BPC + b,
                                      mo * 128:(mo + 1) * 128, :],
                                dst,
                            ))
                    if b == BPC - 2:
                        cur_out_dmas.append(nc.scalar.dma_start(
                            out_d[rep * BPC + b]
                            .rearrange("(m p) c -> p m c", m=MO),
                            o_sb[:, 0:MO * HW],
                        ))
                    elif b % 2 == 1 and b != BPC - 1:
                        cur_out_dmas.append(nc.scalar.dma_start(
                            out_d[rep * BPC + b - 1: rep * BPC + b + 1]
                            .rearrange("b (m p) c -> p b m c", m=MO),
                            o_sb[:],
                        ))
    nc.compile()
    return nc


def _host_prep(routing_weights, weight):
    """Per-core host layouts: W2[(e,s),(ki,t,i)], S[(e,s),(b,s')]."""
    wt2 = np.ascontiguousarray(
        weight.reshape(E, TT, SS, KI, 128).transpose(0, 2, 3, 1, 4)
        .reshape(E * SS, WCOL2).astype(np.float16))
    eye = np.eye(SS, dtype=np.float32)
    sels = []
    for c in range(N_CORES):
        r = routing_weights[c * BPC:(c + 1) * BPC]          # [BPC, E]
        s = np.einsum('be,st->esbt', r, eye)                # [E,S,B,S]
        sels.append(np.ascontiguousarray(
            s.reshape(E * SS, BPC * SS).astype(np.float16)))
    return wt2, sels


def _bench_inputs(rng):
    """Random per-core input map matching _build's dram tensors."""
    return {
        "x": rng.standard_normal((BPC, C_IN, HW),
                                 dtype=np.float32).astype(np.float16),
        "sel": rng.random((128, 128), dtype=np.float32).astype(np.float16),
        "wt2": rng.standard_normal((128, WCOL2),
                                   dtype=np.float32).astype(np.float16),
    }


def kernel(x: np.ndarray, routing_weights: np.ndarray, weight: np.ndarray,
           _trace: bool = False):
    from concourse.bass_utils import run_bass_kernel_spmd

    x = np.asarray(x, dtype=np.float32)
    routing_weights = np.ascontiguousarray(
        np.asarray(routing_weights, dtype=np.float32))
    weight = np.asarray(weight, dtype=np.float32)

    if "nc" not in _cache:
        _cache["nc"] = _build()
    nc = _cache["nc"]

    wt2, sels = _host_prep(routing_weights, weight)
    x_r = np.ascontiguousarray(x.reshape(B, C_IN, HW).astype(np.float16))

    in_maps = []
    for c in range(N_CORES):
        sl = slice(c * BPC, (c + 1) * BPC)
        in_maps.append({"x": x_r[sl], "sel": sels[c], "wt2": wt2})

    res = run_bass_kernel_spmd(nc, in_maps, core_ids=list(range(N_CORES)),
                               trace=_trace)
    out = np.concatenate([res.results[c]["out"] for c in range(N_CORES)],
                         axis=0)
    if _trace:
        _cache["last_result"] = res
    return out.reshape(B, C_OUT, H, W).astype(np.float32)


if __name__ == "__main__":
    rng = np.random.default_rng(0)
    x = rng.standard_normal((B, C_IN, H, W), dtype=np.float32)
    rw = rng.random((B, E), dtype=np.float32)
    w = rng.standard_normal((E, C_OUT, C_IN), dtype=np.float32)
    got = kernel(x, rw, w)
    agg = np.einsum('be,eoi->boi', rw, w)
    want = np.einsum('boi,bihw->bohw', agg, x.reshape(B, C_IN, H, W))
    err = np.abs(got - want).max() / np.abs(want).max()
    print("rel err:", err)
